# revision 1
# baseline (speedup 1.0000x reference)
"""DispersionLoss kernel for Trainium2 (8 NeuronCores, Bass/Tile).

Reference computation (N=16384, F=64, K=32, C=128):
    bin_mass[f,k]  = sum_n m[n,f,k] + EPS
    SWY[f,k,c]     = sum_n m[n,f,k] * y[n,c]
    cent[f,k,c]    = SWY / bin_mass
    loss_dispersion= sum_fk ( sum_n m*dist2 ) / bin_mass
                   = sum_fk ( A/bin_mass - c_sq - EPS*c_sq/bin_mass )
        where A[f,k] = sum_n m[n,f,k]*|y_n|^2   (algebraic expansion: the
        cross term sum_n m*cross equals bin_mass*c_sq exactly)
    loss_entropy   = sum_fk p*log(p+EPS), p = bin_mass/N
    loss_repulsion = sum_f sum_k exp(-|cent[f,k]-cent[f,k+1]|^2)
    loss_inter     = sum_f sum_{k<j} exp(-|cent[f,k]-cent[f,j]|^2) / F
                   = sum_f (sum_{kj} exp(-pairwise) - K) / 2 / F   (symmetry)

Sharding: over F (8 features per core) -> every loss term decomposes per-f,
so no cross-core collectives are needed; host sums 8 partial scalars.

Each core (inputs arrive fp16, host-packed into DMA-friendly layouts):
  phase 1: [Y | 1] resident in SBUF; per 128-row subtile two fp16 matmuls
    accumulate psum_swyT[c=128, fk=256] += Y.T @ G and
    psum_am[2, fk] += [1 | y_sq].T @ G  (y_sq precomputed in batches).
  phase 2: transpose to bin-major, per-bin stats vectorized across both
    128-bin halves, mean-centered all-pairs distance stage (exp on whole
    blocks, diagonal-block reduces), raw sums DMA'd out; the host sums the
    8 cores' partials and applies the final linear combines in fp64.
"""

import numpy as np

N = 16384
F = 64
K = 32
C = 128
NCORES = 8
F_PER_CORE = F // NCORES          # 8
FK = F_PER_CORE * K               # 256 bins per core
NT = N // 128                     # 128 row-tiles

LAMBDA_ENTROPY = 0.1
LAMBDA_REPULSION = 0.5
LAMBDA_INTER = 0.3
EPS = 1e-8

PG = 8                            # n-subtiles per packed G super-tile
NB = NT // PG                     # 16 super-tiles
YW = C + 1                        # 129: [Y | 1]
SQB = 8                           # subtiles per square/reduce batch

_NC_CACHE = {}


def _pack_g(gc: np.ndarray) -> np.ndarray:
    """(N, FK) -> (NB*128, PG*FK): row p of block b holds subtile rows
    [b*PG*128 + t*128 + p for t in range(PG)] concatenated."""
    return np.ascontiguousarray(
        gc.reshape(NB, PG, 128, FK).transpose(0, 2, 1, 3).reshape(NB * 128, PG * FK)
    )


def _pack_y(yo: np.ndarray) -> np.ndarray:
    """(N, YW) -> (128, NT*YW): partition p holds rows [s*128+p for s] concat."""
    return np.ascontiguousarray(
        yo.reshape(NT, 128, YW).transpose(1, 0, 2).reshape(128, NT * YW)
    )


def _finalize(parts: np.ndarray):
    """parts: (ncores, 8) raw per-core sums
    [wv0, wv1, ent0, ent1, en_tot, en_inv, e_allsum, 0]."""
    r = parts.astype(np.float64).sum(axis=0)
    disp = r[0] + r[1]
    ent = r[2] + r[3]
    rep = r[4] - r[5]
    inter = (r[6] - F * K) / (2.0 * F)
    tot = disp + LAMBDA_ENTROPY * ent + LAMBDA_REPULSION * rep + LAMBDA_INTER * inter
    return tuple(np.float32(v) for v in (tot, disp, ent, rep, inter))


def _build_nc(mode: str):
    import concourse.bacc as bacc
    import concourse.tile as tile
    from concourse import mybir

    f32 = mybir.dt.float32
    fin = {"f32": mybir.dt.float32, "f32r": mybir.dt.float32r,
           "f16": mybir.dt.float16}[mode]

    nc = bacc.Bacc("TRN2", target_bir_lowering=False, debug=False,
                   enable_asserts=False, enable_partition_id=False)
    # host-packed layouts (see _pack_g/_pack_y)
    g_dram = nc.dram_tensor("g", (NB * 128, PG * FK), fin, kind="ExternalInput").ap()
    y_dram = nc.dram_tensor("y", (128, NT * YW), fin, kind="ExternalInput").ap()
    out_dram = nc.dram_tensor("out", (1, 8), f32, kind="ExternalOutput").ap()

    with tile.TileContext(nc) as tc:
        with (
            tc.tile_pool(name="singles", bufs=1) as singles,
            tc.tile_pool(name="gpool", bufs=8) as gpool,
            tc.tile_pool(name="scr", bufs=2) as scr,
            tc.tile_pool(name="ph2", bufs=1) as ph2,
            tc.tile_pool(name="psacc", bufs=1, space="PSUM") as psacc,
            tc.tile_pool(name="pstmp", bufs=2, space="PSUM") as pstmp,
        ):
            # ---- constants ----
            mi2 = singles.tile([128, 128], f32)          # -2 * identity
            nc.gpsimd.memset(mi2, 0.0)
            nc.gpsimd.affine_select(
                out=mi2, in_=mi2,
                compare_op=mybir.AluOpType.not_equal,
                fill=-2.0, base=0, pattern=[[-1, 128]], channel_multiplier=1,
            )
            ones128 = singles.tile([128, 1], f32)
            nc.gpsimd.memset(ones128, 1.0)
            onesrow = singles.tile([1, 128], f32)
            nc.gpsimd.memset(onesrow, 1.0)
            eps128 = singles.tile([128, 1], f32)
            nc.gpsimd.memset(eps128, EPS)

            id128 = singles.tile([128, 128], f32)        # +identity
            nc.gpsimd.memset(id128, 0.0)
            nc.gpsimd.affine_select(
                out=id128, in_=id128,
                compare_op=mybir.AluOpType.not_equal,
                fill=1.0, base=0, pattern=[[-1, 128]], channel_multiplier=1,
            )

            # ---- [Y | 1] resident (128 x NT*YW); chunks DMA'd on the
            # scalar queue, interleaved with the main loop so the cold-start
            # backlog stays small.  Chunk j covers subtiles 16j..16j+15.
            yres = singles.tile([128, NT * YW], fin, name="yres")
            # yqt holds per-subtile extras stationaries [1 | y_sq] at cols
            # (2s, 2s+1).  Even cols = 1.0 (one ACT const-fill), odd cols =
            # y_sq computed in SQB-subtile batches.
            yqt = singles.tile([128, 2 * NT], fin, name="yqt")
            yqt3 = yqt.rearrange("p (t two) -> p t two", two=2)

            def emit_square_batch(s0, nb):
                    sqs = scr.tile([128, SQB * YW], f32, tag="sqs", name="sqs")
                    nc.scalar.activation(
                        out=sqs[:, 0:nb * YW], in_=yres[:, s0 * YW:(s0 + nb) * YW],
                        func=mybir.ActivationFunctionType.Square,
                    )
                    red = scr.tile([128, SQB], f32, tag="red", name="red")
                    nc.vector.reduce_sum(
                        red[:, 0:nb],
                        sqs[:, 0:nb * YW].rearrange(
                            "p (t c) -> p t c", c=YW)[:, :, 0:C],
                        axis=mybir.AxisListType.X,
                    )
                    with nc.allow_low_precision(reason="y_sq feeds f32r mm"):
                        nc.vector.tensor_copy(
                            out=yqt3[:, s0:s0 + nb, 1:2],
                            in_=red[:, 0:nb].rearrange(
                                "p (t one) -> p t one", one=1),
                        )

            CHUNKS = [(0, 4), (4, 16), (16, 32), (32, 48), (48, 64),
                      (64, 80), (80, 96), (96, 112), (112, 128)]

            def emit_ychunk(lo, hi):
                nc.scalar.dma_start(
                    out=yres[:, lo * YW:hi * YW],
                    in_=y_dram[:, lo * YW:hi * YW],
                )

            def emit_squares_rng(lo, hi):
                for s0 in range(lo, hi, SQB):
                    emit_square_batch(s0, min(SQB, hi - s0))

            emit_ychunk(*CHUNKS[0])
            with nc.allow_low_precision(reason="extras feed f32r matmul"):
                nc.scalar.activation(
                    out=yqt3[:, :, 0:1],
                    in_=yres[:, 0:NT].rearrange("p (t one) -> p t one", one=1),
                    func=mybir.ActivationFunctionType.Copy,
                    scale=0.0, bias=1.0,
                )
            emit_squares_rng(*CHUNKS[0])
            for lo, hi in CHUNKS[1:]:
                emit_ychunk(lo, hi)
                emit_squares_rng(lo, hi)
            chunk_at_block = {}

            # prefetch the Exp table into its ACT slot right after the last
            # square batch (Ln uses the other slot), so the tail's exps skip
            # the 1.3us table load.
            expwarm = ph2.tile([1, 1], f32)
            nc.scalar.activation(
                out=expwarm, in_=eps128[0:1, 0:1],
                func=mybir.ActivationFunctionType.Exp,
            )

            # ---- phase 1: [Y | 1 | y_sq]^T @ G accumulated over subtiles ----
            # Y is stationary; G streams 256 columns so f32r runs at full
            # rate.  Output layout: (c x fk) + (2 x fk).
            ps_swyT = psacc.tile([128, FK], f32)
            ps_am = psacc.tile([2, FK], f32)
            for b in range(NB):
                if b in chunk_at_block:
                    lo, hi = chunk_at_block[b]
                    emit_ychunk(lo, hi)
                    emit_squares_rng(lo, hi)
                g = gpool.tile([128, PG * FK], fin)
                nc.sync.dma_start(out=g, in_=g_dram[b * 128:(b + 1) * 128, :])
                for t in range(PG):
                    s = b * PG + t
                    rhs = g[:, t * FK:(t + 1) * FK]
                    nc.tensor.matmul(
                        ps_swyT, yres[:, s * YW:s * YW + C], rhs,
                        start=(s == 0), stop=(s == NT - 1),
                    )
                    # extras rows: [1 ; y_sq] -> ps_am rows [mass ; A]
                    nc.tensor.matmul(
                        ps_am, yqt[:, 2 * s:2 * s + 2], rhs,
                        start=(s == 0), stop=(s == NT - 1),
                    )

            # ---- transpose back to (fk x [c | mass | A]) layout ----
            swyT_sb = ph2.tile([128, FK], f32)
            nc.scalar.copy(swyT_sb, ps_swyT)
            am_sb = ph2.tile([2, FK], f32)
            nc.scalar.copy(am_sb, ps_am)
            ps = []
            for h in range(2):
                cs = h * 128
                ps_h = psacc.tile([128, 130], f32, tag=f"ps{h}", name=f"ps{h}")
                nc.tensor.matmul(ps_h[:, 0:C], swyT_sb[:, cs:cs + 128], id128,
                                 start=True, stop=True)
                nc.tensor.matmul(ps_h[:, C:C + 2], am_sb[0:2, cs:cs + 128],
                                 id128[0:2, 0:2], start=True, stop=True)
                ps.append(ps_h)

            # ---- per-bin stats, halves vectorized as columns (128 x 2) ----
            mass2 = ph2.tile([128, 2], f32)
            a2 = ph2.tile([128, 2], f32)
            for h in range(2):
                nc.scalar.activation(
                    out=mass2[:, h:h + 1], in_=ps[h][:, 128:129],
                    func=mybir.ActivationFunctionType.Identity,
                    bias=eps128, scale=1.0,
                )
                nc.vector.tensor_copy(a2[:, h:h + 1], ps[h][:, 129:130])
            inv2 = ph2.tile([128, 2], f32)
            nc.vector.reciprocal(inv2, mass2)
            cent = ph2.tile([128, FK], f32)
            for h in range(2):
                nc.vector.tensor_scalar_mul(
                    cent[:, h * 128:(h + 1) * 128],
                    in0=ps[h][:, 0:C], scalar1=inv2[:, h:h + 1],
                )
            csq_scr = scr.tile([128, FK], f32, tag="csqscr")
            nc.vector.tensor_mul(csq_scr, cent, cent)
            c_sq2 = ph2.tile([128, 2], f32)
            nc.vector.reduce_sum(
                c_sq2, csq_scr.rearrange("p (h c) -> p h c", c=128),
                axis=mybir.AxisListType.X,
            )
            # wv = A*inv - c_sq - EPS*c_sq*inv ; ent = p*ln(p+EPS)
            st = ph2.tile([128, 4], f32)
            t0 = ph2.tile([128, 2], f32)
            nc.vector.tensor_mul(t0, a2, inv2)
            nc.vector.tensor_sub(st[:, 0:2], t0, c_sq2)
            pp2 = ph2.tile([128, 2], f32)
            nc.scalar.mul(pp2, mass2, 1.0 / N)
            lg2 = ph2.tile([128, 2], f32)
            nc.scalar.activation(
                out=lg2, in_=pp2,
                func=mybir.ActivationFunctionType.Ln,
                bias=eps128, scale=1.0,
            )
            nc.vector.tensor_mul(st[:, 2:4], pp2, lg2)
            ps_st = pstmp.tile([1, 4], f32, tag="pstmp")
            nc.tensor.matmul(ps_st, ones128, st, start=True, stop=True)

            # ---- cc = centT - cent_bin0 (c x fk), centering fused into the
            # psum->sbuf copy as a per-partition bias.  Distances are
            # shift-invariant; small operands kill the csq+csq-2dots
            # cancellation.
            cc = ph2.tile([128, FK], f32)
            nshift = ph2.tile([128, 1], f32)
            for h in range(2):
                cs = h * 128
                ps_ct = pstmp.tile([128, 128], f32, tag="pstmp", name=f"pc{h}")
                nc.tensor.matmul(ps_ct, cent[:, cs:cs + 128], id128,
                                 start=True, stop=True)
                if h == 0:
                    nc.vector.tensor_scalar_mul(nshift, in0=ps_ct[:, 0:1],
                                                scalar1=-1.0)
                nc.scalar.activation(
                    out=cc[:, cs:cs + 128], in_=ps_ct,
                    func=mybir.ActivationFunctionType.Identity,
                    bias=nshift, scale=1.0,
                )
            cc2s = scr.tile([128, FK], f32, tag="cc2s")
            nc.vector.tensor_mul(cc2s, cc, cc)
            ps_ccr = pstmp.tile([1, FK], f32, tag="pstmp")
            nc.tensor.matmul(ps_ccr, ones128, cc2s, start=True, stop=True)
            ccr_sb = ph2.tile([1, FK], f32)
            nc.scalar.copy(ccr_sb, ps_ccr)
            cq2 = ph2.tile([128, 2], f32)
            for h in range(2):
                ps_cq = pstmp.tile([128, 1], f32, tag="pstmp", name=f"pq{h}")
                nc.tensor.matmul(ps_cq, ccr_sb[0:1, h * 128:(h + 1) * 128],
                                 ones128[0:1, 0:1], start=True, stop=True)
                nc.scalar.copy(cq2[:, h:h + 1], ps_cq)
            # sct5 columns: [1, ccsq_h0, 1, ccsq_h1, 1]
            sct5 = ph2.tile([128, 5], f32)
            nc.vector.tensor_copy(sct5[:, 0:1], ones128)
            nc.vector.tensor_copy(sct5[:, 1:2], cq2[:, 0:1])
            nc.vector.tensor_copy(sct5[:, 2:3], ones128)
            nc.vector.tensor_copy(sct5[:, 3:4], cq2[:, 1:2])
            nc.vector.tensor_copy(sct5[:, 4:5], ones128)
            top = ph2.tile([2, FK], f32)         # [1 ; c_sq]
            bot = ph2.tile([2, FK], f32)         # [-c_sq/2 ; -1/2]
            for h in range(2):
                cs = h * 128
                ps_t2 = pstmp.tile([2, 128], f32, tag="pstmp", name=f"pt{h}")
                nc.tensor.matmul(ps_t2, sct5[:, 2 * h:2 * h + 2], mi2,
                                 start=True, stop=True)
                nc.scalar.mul(top[0:2, cs:cs + 128], ps_t2, -0.5)
                ps_b2 = pstmp.tile([2, 128], f32, tag="pstmp", name=f"pb{h}")
                nc.tensor.matmul(ps_b2, sct5[:, 2 * h + 1:2 * h + 3], mi2,
                                 start=True, stop=True)
                nc.scalar.mul(bot[0:2, cs:cs + 128], ps_b2, 0.25)

            # ---- repulsion: adjacent-bin distances from cc ----
            dd = ph2.tile([128, FK - 1], f32)
            nc.vector.tensor_sub(dd, cc[:, 0:FK - 1], cc[:, 1:FK])
            nc.vector.tensor_mul(dd, dd, dd)
            ps_nd = pstmp.tile([1, FK - 1], f32, tag="pstmp")
            nc.tensor.matmul(ps_nd, ones128, dd, start=True, stop=True)
            en = ph2.tile([1, FK - 1], f32)
            en_tot = ph2.tile([1, 1], f32)
            nc.scalar.activation(
                out=en, in_=ps_nd, func=mybir.ActivationFunctionType.Exp,
                scale=-1.0, accum_out=en_tot,
            )
            inv_view = en[0:1, 0:(F_PER_CORE - 1) * K].rearrange(
                "p (a b) -> p a b", b=K
            )[:, :, K - 1:K]
            inv_sum = ph2.tile([1, 1], f32)
            nc.vector.reduce_sum(inv_sum, inv_view, axis=mybir.AxisListType.XY)

            # ---- inter: psq[k,j] = dots - (c_sq[k]+c_sq[j])/2 for ALL global
            # pairs; E = exp(2*psq) of the whole block (all entries are valid
            # distances, no overflow), then DVE-reduce only the diagonal
            # (same-f) blocks.
            erows = ph2.tile([128, 2], f32)
            for q in range(2):
                psq = pstmp.tile([128, FK], f32, tag="pwq", name=f"psq{q}")
                nc.tensor.matmul(psq, cc[:, q * 128:(q + 1) * 128], cc,
                                 start=True, stop=False)
                nc.tensor.matmul(psq, top[:, q * 128:(q + 1) * 128], bot,
                                 start=False, stop=True)
                e_full = scr.tile([128, FK], f32, tag="efull", name=f"ef{q}")
                nc.scalar.activation(
                    out=e_full, in_=psq,
                    func=mybir.ActivationFunctionType.Exp, scale=2.0,
                )
                for fl in range(4):
                    fg = q * 4 + fl
                    nc.vector.reduce_sum(
                        erows[32 * fl:32 * fl + 32, q:q + 1],
                        e_full[32 * fl:32 * fl + 32, fg * 32:fg * 32 + 32],
                        axis=mybir.AxisListType.X,
                    )
            ecol = ph2.tile([128, 1], f32)
            nc.vector.reduce_sum(ecol, erows, axis=mybir.AxisListType.X)
            ps_i = pstmp.tile([1, 1], f32, tag="pstmp")
            nc.tensor.matmul(ps_i, ones128, ecol, start=True, stop=True)

            # ---- raw outputs; host finishes the linear combines ----
            # res = [wv0, wv1, ent0, ent1, en_tot, en_inv, e_allsum, 0]
            res = ph2.tile([1, 8], f32)
            nc.gpsimd.memset(res, 0.0)
            nc.scalar.copy(res[0:1, 0:4], ps_st)
            nc.vector.tensor_copy(res[0:1, 4:5], en_tot)
            nc.vector.tensor_copy(res[0:1, 5:6], inv_sum)
            nc.scalar.copy(res[0:1, 6:7], ps_i)
            nc.sync.dma_start(out=out_dram, in_=res)

    nc.compile()
    return nc


def get_nc(mode: str = "f16"):
    if mode not in _NC_CACHE:
        _NC_CACHE[mode] = _build_nc(mode)
    return _NC_CACHE[mode]


def kernel(membership: np.ndarray, teacher_preds: np.ndarray, _trace: bool = False,
           _mode: str = "f16"):
    from concourse.bass_utils import run_bass_kernel_spmd

    np_in = np.float16 if _mode == "f16" else np.float32
    m = np.asarray(membership, dtype=np_in).reshape(N, F * K)
    y = np.asarray(teacher_preds, dtype=np_in)
    y = _pack_y(np.concatenate([y, np.ones((N, 1), dtype=np_in)], axis=1))

    nc = get_nc(_mode)
    in_maps = []
    for i in range(NCORES):
        in_maps.append({
            "g": _pack_g(m[:, i * FK:(i + 1) * FK]),
            "y": y,
        })
    res = run_bass_kernel_spmd(
        nc, in_maps, core_ids=list(range(NCORES)), trace=_trace,
    )
    parts = np.stack(
        [np.asarray(res.results[i]["out"][0], dtype=np.float64) for i in range(NCORES)]
    )
    out = _finalize(parts)
    if _trace:
        return out, res
    return out


if __name__ == "__main__":
    rng = np.random.default_rng(0)
    mem = rng.random((N, F, K), dtype=np.float32)
    tp = rng.random((N, C), dtype=np.float32)
    print(kernel(mem, tp))



# revision 15
# speedup vs baseline: 1.5261x; 1.5261x over previous
"""DispersionLoss kernel for Trainium2 (8 NeuronCores, Bass/Tile).

Reference computation (N=16384, F=64, K=32, C=128):
    bin_mass[f,k]  = sum_n m[n,f,k] + EPS
    SWY[f,k,c]     = sum_n m[n,f,k] * y[n,c]
    cent[f,k,c]    = SWY / bin_mass
    loss_dispersion= sum_fk (A/bin_mass - c_sq)   [EPS*c_sq/bin_mass term ~1e-11, dropped]
        where A[f,k] = sum_n m[n,f,k]*|y_n|^2
    loss_entropy   = sum_fk p*log(p+EPS), p = bin_mass/N
    loss_repulsion = sum_f sum_k exp(-|cent[f,k]-cent[f,k+1]|^2)
    loss_inter     = sum_f (sum_{kj} exp(-pairwise) - K) / 2 / F

Sharding: over F (8 features per core) -> every loss term decomposes per-f,
no cross-core collectives; host sums 8 partial vectors.

v2 design (vs v1 Y-stationary fp16):
  - inputs quantized to fp8 e4m3 on host; ysq precomputed on host in f32 and
    stored as hi+lo fp8 pair -> device does zero prep work.  DMA: 6.1 MiB/core.
  - G-stationary DoubleRow matmuls: per 256-row pair u and bin-half h,
    psum[fk=128, 132] += g[:, u, :, h*128:+128].T @ [Y | 1 | ysq_h | ysq_l].
    G enters the PE once; mass/A ride along as columns 128..130; output is
    bin-major so the tail needs no transpose stage.
  - tail: per-bin stats are per-partition vector ops; pairwise stage uses
    centered fp16 centroids (2 transposes + fp16 matmuls); Exp/Ln tables
    preloaded at t~=3us so the tail has no ACT_TABLE_LOAD stalls.
"""

import numpy as np

N = 16384
F = 64
K = 32
C = 128
NCORES = 8
F_PER_CORE = F // NCORES          # 8
FK = F_PER_CORE * K               # 256 bins per core
NPAIR = N // 256                  # 64 subtile pairs (DoubleRow: 256 rows/mm)
W = 132                           # moving cols: [y(128) | 1 | ysq_h | ysq_l | pad]
GB = 8                            # pairs per g DMA block
NGB = NPAIR // GB                 # 8 g blocks
YCH = 16                          # pairs per y DMA chunk
NYCH = NPAIR // YCH               # 4 y chunks

LAMBDA_ENTROPY = 0.1
LAMBDA_REPULSION = 0.5
LAMBDA_INTER = 0.3
EPS = 1e-8

USE_DOUBLE_ROW = True

_NC_CACHE = {}


def _f8dtype():
    import ml_dtypes
    return ml_dtypes.float8_e4m3


def _pack_g(gc: np.ndarray) -> np.ndarray:
    """(N, FK) fp8 -> (NGB*128, GB*2*FK): block b row p holds, for the 8
    pairs u of the block, [i=0 rows | i=1 rows] x FK cols where the n-row is
    256*u + 128*i + p."""
    x = gc.reshape(NPAIR, 2, 128, FK).transpose(2, 0, 1, 3)   # p, u, i, fk
    x = x.reshape(128, NPAIR * 2 * FK).reshape(128, NGB, GB * 2 * FK)
    return np.ascontiguousarray(x.transpose(1, 0, 2).reshape(NGB * 128, GB * 2 * FK))


def _pack_y(yslab: np.ndarray) -> np.ndarray:
    """(N, W) fp8 -> (128, NPAIR*2*W): partition p holds pair-major slabs."""
    return np.ascontiguousarray(
        yslab.reshape(NPAIR, 2, 128, W).transpose(2, 0, 1, 3).reshape(128, NPAIR * 2 * W)
    )


def _finalize(parts: np.ndarray):
    """parts: (ncores, 8) = [wv0, wv1, mlg0, mlg1, eall0, eall1, rep0, rep1]."""
    r = parts.astype(np.float64).sum(axis=0)
    disp = r[0] + r[1]
    ent = (r[2] + r[3]) / N
    rep = r[6] + r[7]
    inter = (r[4] + r[5] - F * K) / (2.0 * F)
    tot = disp + LAMBDA_ENTROPY * ent + LAMBDA_REPULSION * rep + LAMBDA_INTER * inter
    return tuple(np.float32(v) for v in (tot, disp, ent, rep, inter))


def _build_nc():
    import concourse.bacc as bacc
    import concourse.tile as tile
    from concourse import mybir

    f32 = mybir.dt.float32
    f16 = mybir.dt.float16
    f8 = mybir.dt.float8e4
    DR = mybir.MatmulPerfMode.DoubleRow
    AF = mybir.ActivationFunctionType

    nc = bacc.Bacc("TRN2", target_bir_lowering=False, debug=False,
                   enable_asserts=False, enable_partition_id=False)
    g_dram = nc.dram_tensor("g", (NGB * 128, GB * 2 * FK), f8, kind="ExternalInput").ap()
    y_dram = nc.dram_tensor("y", (128, NPAIR * 2 * W), f8, kind="ExternalInput").ap()
    out_dram = nc.dram_tensor("out", (1, 8), f32, kind="ExternalOutput").ap()

    with tile.TileContext(nc) as tc:
        with (
            tc.tile_pool(name="singles", bufs=1) as singles,
            tc.tile_pool(name="gpool", bufs=4) as gpool,
            tc.tile_pool(name="scr", bufs=2) as scr,
            tc.tile_pool(name="ph2", bufs=1) as ph2,
            tc.tile_pool(name="psacc", bufs=1, space="PSUM") as psacc,
            tc.tile_pool(name="pstmp", bufs=1, space="PSUM") as pstmp,
        ):
            # ---- y stream: 4 chunks split across gpsimd/vector queues so the
            # scalar engine is free for the table preloads.
            yres = singles.tile([128, NPAIR * 2 * W], f8, name="yres")
            for ci in range(NYCH):
                lo = ci * YCH * 2 * W
                hi = (ci + 1) * YCH * 2 * W
                eng = nc.gpsimd if ci % 2 == 0 else nc.scalar
                eng.dma_start(out=yres[:, lo:hi], in_=y_dram[:, lo:hi])

            # ---- constants ----
            ones128 = singles.tile([128, 1], f32)
            nc.gpsimd.memset(ones128, 1.0)
            eps128 = singles.tile([128, 1], f32)
            nc.gpsimd.memset(eps128, EPS)
            id16 = singles.tile([128, 128], f16)
            nc.gpsimd.memset(id16, 0.0)
            nc.gpsimd.affine_select(
                out=id16, in_=id16,
                compare_op=mybir.AluOpType.not_equal,
                fill=1.0, base=0, pattern=[[-1, 128]], channel_multiplier=1,
            )
            # rank-1 operand rows for the pairwise stage (partition 0 only)
            ones_row = singles.tile([1, FK], f16)
            nc.gpsimd.memset(ones_row, 1.0)
            qneg_sb = singles.tile([1, FK], f16)
            # repulsion masks: Mrep_h[p, j] = 1 iff j == 128h + p + 1 and the
            # pair does not cross a feature boundary ((128h+p) % 32 != 31).
            # Fill via a 2D-pattern affine_select on the [128, 8, 31] subview
            # (columns j%32 != 0), so crossings are never touched:
            # fill where 32*blk + c' - p - 128h == 0 (j = 32*blk + c' + 1).
            mrep = []
            for h in range(2):
                m = singles.tile([128, FK], f32, name=f"mrep{h}")
                nc.gpsimd.memset(m, 0.0)
                m3 = m.rearrange("p (blk c) -> p blk c", c=32)
                nc.gpsimd.affine_select(
                    out=m3[:, :, 1:32], in_=m3[:, :, 1:32],
                    compare_op=mybir.AluOpType.not_equal,
                    fill=1.0, base=-128 * h, pattern=[[32, 8], [1, 31]],
                    channel_multiplier=-1,
                )
                mrep.append(m)

            # ---- preload Exp+Ln activation tables (2 slots) so the tail has
            # no 1.5us ACT_TABLE_LOAD stalls.
            warm = ph2.tile([1, 2], f32)
            nc.scalar.activation(out=warm[0:1, 0:1], in_=ones128[0:1, 0:1], func=AF.Exp)
            nc.scalar.activation(out=warm[0:1, 1:2], in_=ones128[0:1, 0:1], func=AF.Ln)

            # ---- phase 1: G-stationary DoubleRow accumulation ----
            # ps[h][fk_local, 0:128]=SWY, [:,128]=mass_raw, [:,129:131]=A_hi/lo
            ps = [psacc.tile([128, W], f32, name=f"acc{h}") for h in range(2)]
            for b in range(NGB):
                g = gpool.tile([128, GB * 2 * FK], f8)
                nc.sync.dma_start(out=g, in_=g_dram[b * 128:(b + 1) * 128, :])
                for ul in range(GB):
                    u = b * GB + ul
                    if USE_DOUBLE_ROW:
                        gv = g[:, ul * 2 * FK:(ul + 1) * 2 * FK].rearrange(
                            "p (i fk) -> p i fk", i=2)
                        yv = yres[:, u * 2 * W:(u + 1) * 2 * W].rearrange(
                            "p (i w) -> p i w", i=2)
                        for h in range(2):
                            nc.tensor.matmul(
                                ps[h], gv[:, :, h * 128:(h + 1) * 128], yv,
                                start=(u == 0), stop=(u == NPAIR - 1),
                                perf_mode=DR,
                            )
                    else:
                        for i in range(2):
                            s = 2 * u + i
                            sl = 2 * ul + i
                            yv = yres[:, s * W:(s + 1) * W]
                            for h in range(2):
                                nc.tensor.matmul(
                                    ps[h],
                                    g[:, sl * FK + h * 128:sl * FK + (h + 1) * 128],
                                    yv,
                                    start=(s == 0), stop=(s == 2 * NPAIR - 1),
                                )

            # ---- tail ----
            # st cols: [wv0, wv1, mass*ln0, mass*ln1, eall0, eall1, rep0, rep1]
            st = ph2.tile([128, 8], f32)
            mass = ph2.tile([128, 2], f32)
            inv = ph2.tile([128, 2], f32)
            a_ = ph2.tile([128, 2], f32)
            s2 = ph2.tile([128, 2], f32)
            csq = ph2.tile([128, 2], f32)
            t1 = ph2.tile([128, 2], f32)
            lg = ph2.tile([128, 2], f32)
            cent16 = ph2.tile([128, 2 * 128], f16)
            ccT = ph2.tile([128, FK], f16)
            nshift = ph2.tile([128, 1], f32)
            sqf = scr.tile([128, FK], f32, tag="sqf")
            qrow_sb = ph2.tile([1, FK], f32)

            psT = []
            for h in range(2):
                hs = slice(h, h + 1)
                nc.scalar.activation(out=mass[:, hs], in_=ps[h][:, 128:129],
                                     func=AF.Identity, bias=eps128, scale=1.0)
                # A = A_hi + A_lo (single-PSUM-operand reduce over 2 cols)
                nc.vector.reduce_sum(
                    a_[:, hs],
                    ps[h][:, 129:131].rearrange("p (one c) -> p one c", one=1),
                    axis=mybir.AxisListType.X,
                )
                nc.vector.reciprocal(inv[:, hs], mass[:, hs])
                # cent16 = SWY * inv (fp16, feeds the pairwise stage)
                with nc.allow_low_precision(reason="cent fp16 for exp terms"):
                    nc.vector.tensor_scalar_mul(
                        cent16[:, h * 128:(h + 1) * 128],
                        in0=ps[h][:, 0:128], scalar1=inv[:, hs],
                    )
                # csq = sum_c cent16^2
                sq_scr = scr.tile([128, 128], f32, tag="sqscr", name=f"sqs{h}")
                nc.vector.tensor_mul(sq_scr, cent16[:, h * 128:(h + 1) * 128],
                                     cent16[:, h * 128:(h + 1) * 128])
                nc.vector.reduce_sum(csq[:, hs], sq_scr, axis=mybir.AxisListType.X)
                # wv = A*inv - csq
                nc.vector.tensor_mul(t1[:, hs], a_[:, hs], inv[:, hs])
                nc.vector.tensor_sub(st[:, h:h + 1], t1[:, hs], csq[:, hs])
                # entropy: st2/3 = mass * ln(mass/N + EPS); host divides by N
                nc.scalar.activation(out=lg[:, hs], in_=mass[:, hs],
                                     func=AF.Ln, bias=eps128, scale=1.0 / N)
                nc.vector.tensor_mul(st[:, 2 + h:3 + h], mass[:, hs], lg[:, hs])
                # transpose to [c, fk_half]
                ps_t = pstmp.tile([128, 128], f32, tag=f"psT{h}", name=f"psT{h}")
                nc.tensor.matmul(ps_t, cent16[:, h * 128:(h + 1) * 128], id16,
                                 start=True, stop=True)
                psT.append(ps_t)

            # center by global bin 0 during the psum->sbuf copy (fp16 out)
            nc.vector.tensor_scalar_mul(nshift, in0=psT[0][:, 0:1], scalar1=-1.0)
            for h in range(2):
                with nc.allow_low_precision(reason="cc fp16 for exp terms"):
                    nc.scalar.activation(
                        out=ccT[:, h * 128:(h + 1) * 128], in_=psT[h],
                        func=AF.Identity, bias=nshift, scale=1.0,
                    )
            # q_j = sum_c ccT^2 (column sums via ones-matmul)
            nc.vector.tensor_mul(sqf, ccT, ccT)
            ps_q = pstmp.tile([1, FK], f32, tag="psq")
            nc.tensor.matmul(ps_q, ones128, sqf, start=True, stop=True)
            with nc.allow_low_precision(reason="q fp16 rank-1 operands"):
                nc.scalar.activation(out=qneg_sb, in_=ps_q, func=AF.Copy, scale=-0.5)

            # pairwise: psE = dots - q_k/2 - q_j/2 ; E = exp(2*psE)
            for h in range(2):
                ps_e = pstmp.tile([128, FK], f32, tag=f"psE{h}", name=f"psE{h}")
                nc.tensor.matmul(ps_e, ccT[:, h * 128:(h + 1) * 128], ccT,
                                 start=True, stop=False)
                nc.tensor.matmul(ps_e, ones_row[0:1, 0:128], qneg_sb,
                                 start=False, stop=False)
                nc.tensor.matmul(ps_e, qneg_sb[0:1, h * 128:(h + 1) * 128], ones_row,
                                 start=False, stop=True)
                e_full = scr.tile([128, FK], f32, tag="efull", name=f"ef{h}")
                nc.scalar.activation(out=e_full, in_=ps_e, func=AF.Exp, scale=2.0)
                # repulsion: masked sum (superdiagonal within features)
                edump = scr.tile([128, FK], f32, tag="edump", name=f"ed{h}")
                nc.vector.tensor_mul(edump, e_full, mrep[h])
                nc.vector.reduce_sum(st[:, 6 + h:7 + h], edump,
                                     axis=mybir.AxisListType.X)
                # inter: same-feature 32x32 block row-sums
                for fl in range(4):
                    fg = h * 4 + fl
                    nc.vector.reduce_sum(
                        st[32 * fl:32 * fl + 32, 4 + h:5 + h],
                        e_full[32 * fl:32 * fl + 32, fg * 32:fg * 32 + 32],
                        axis=mybir.AxisListType.X,
                    )

            ps_res = pstmp.tile([1, 8], f32, tag="psres")
            nc.tensor.matmul(ps_res, ones128, st, start=True, stop=True)
            res = ph2.tile([1, 8], f32)
            nc.scalar.copy(res, ps_res)
            nc.sync.dma_start(out=out_dram, in_=res)

    nc.compile()
    return nc


def get_nc():
    if "v2" not in _NC_CACHE:
        _NC_CACHE["v2"] = _build_nc()
    return _NC_CACHE["v2"]


def kernel(membership: np.ndarray, teacher_preds: np.ndarray, _trace: bool = False):
    from concourse.bass_utils import run_bass_kernel_spmd

    f8 = _f8dtype()
    m = np.asarray(membership, dtype=np.float32).reshape(N, F * K)
    y32 = np.asarray(teacher_preds, dtype=np.float32)
    ysq = np.einsum("nc,nc->n", y32.astype(np.float64), y32.astype(np.float64))
    ysq = ysq.astype(np.float32)
    ysq_h = ysq.astype(f8)
    ysq_l = (ysq - ysq_h.astype(np.float32)).astype(f8)
    yslab = np.zeros((N, W), dtype=f8)
    yslab[:, 0:C] = y32.astype(f8)
    yslab[:, C] = np.float32(1.0)
    yslab[:, C + 1] = ysq_h
    yslab[:, C + 2] = ysq_l
    ypacked = _pack_y(yslab)

    m8 = m.astype(f8)
    nc = get_nc()
    in_maps = []
    for i in range(NCORES):
        in_maps.append({
            "g": _pack_g(m8[:, i * FK:(i + 1) * FK]),
            "y": ypacked,
        })
    res = run_bass_kernel_spmd(
        nc, in_maps, core_ids=list(range(NCORES)), trace=_trace,
    )
    parts = np.stack(
        [np.asarray(res.results[i]["out"][0], dtype=np.float64) for i in range(NCORES)]
    )
    out = _finalize(parts)
    if _trace:
        return out, res
    return out


if __name__ == "__main__":
    rng = np.random.default_rng(0)
    mem = rng.random((N, F, K), dtype=np.float32)
    tp = rng.random((N, C), dtype=np.float32)
    print(kernel(mem, tp))


# revision 17
# speedup vs baseline: 1.7315x; 1.1346x over previous
"""DispersionLoss kernel for Trainium2 (8 NeuronCores, Bass/Tile).

Reference computation (N=16384, F=64, K=32, C=128):
    bin_mass[f,k]  = sum_n m[n,f,k] + EPS
    SWY[f,k,c]     = sum_n m[n,f,k] * y[n,c]
    cent[f,k,c]    = SWY / bin_mass
    loss_dispersion= sum_fk (A/bin_mass - c_sq)   [EPS*c_sq/bin_mass ~1e-11, dropped]
        where A[f,k] = sum_n m[n,f,k]*|y_n|^2
    loss_entropy   = sum_fk p*log(p+EPS), p = bin_mass/N
    loss_repulsion = sum_f sum_k exp(-|cent[f,k]-cent[f,k+1]|^2)
    loss_inter     = sum_f (sum_{kj} exp(-pairwise) - K) / 2 / F

Sharding: over F (8 features per core) -> every loss term decomposes per-f,
no cross-core collectives; host sums 8 partial vectors.

Design:
  - inputs quantized to fp8 e4m3 on host; ysq precomputed on host (f32) and
    shipped as an fp8 hi+lo pair -> device does zero prep work.  ~6.1 MiB/core.
  - all input DMA on the sync queue in consumption order (y chunks
    interleaved between g blocks); gpsimd's software-DGE path is avoided.
  - G-stationary DoubleRow fp8 matmuls: for each 256-row pair u and bin-half
    h, psum[:, h*W:(h+1)*W] += g[u,:,h-half].T @ [Y | 1 | ysq_h | ysq_l].
    G enters the PE once; mass/A ride along as extra moving columns; output
    is bin-major so no transpose stage is needed for the per-bin stats.
  - tail: only Ln/Exp touch the scalar engine (tables preloaded at start; no
    ACT_TABLE_LOAD stalls); everything else on vector/gpsimd/PE.  Centroids
    are centered and scaled x16 so all fp16 pairwise math is in normal range.
"""

import numpy as np

N = 16384
F = 64
K = 32
C = 128
NCORES = 8
F_PER_CORE = F // NCORES          # 8
FK = F_PER_CORE * K               # 256 bins per core
NPAIR = N // 256                  # 64 subtile pairs (DoubleRow: 256 rows/mm)
W = 132                           # moving cols: [y(128) | 1 | ysq_h | ysq_l | pad]
GB = 8                            # pairs per g DMA block
NGB = NPAIR // GB                 # 8 g blocks
YCH = 16                          # pairs per y DMA chunk
NYCH = NPAIR // YCH               # 4 y chunks
CSC = 16.0                        # centered-centroid scale (keeps fp16 normal)

LAMBDA_ENTROPY = 0.1
LAMBDA_REPULSION = 0.5
LAMBDA_INTER = 0.3
EPS = 1e-8

_NC_CACHE = {}


def _f8dtype():
    import ml_dtypes
    return ml_dtypes.float8_e4m3


def _pack_g(gc: np.ndarray) -> np.ndarray:
    """(N, FK) fp8 -> (NGB*128, GB*2*FK): block b row p holds, for the 8
    pairs u of the block, [i=0 | i=1] x FK cols where the n-row is
    256*u + 128*i + p."""
    x = gc.reshape(NPAIR, 2, 128, FK).transpose(2, 0, 1, 3)   # p, u, i, fk
    x = x.reshape(128, NPAIR * 2 * FK).reshape(128, NGB, GB * 2 * FK)
    return np.ascontiguousarray(x.transpose(1, 0, 2).reshape(NGB * 128, GB * 2 * FK))


def _pack_y(yslab: np.ndarray) -> np.ndarray:
    """(N, W) fp8 -> (128, NPAIR*2*W): partition p holds pair-major slabs."""
    return np.ascontiguousarray(
        yslab.reshape(NPAIR, 2, 128, W).transpose(2, 0, 1, 3).reshape(128, NPAIR * 2 * W)
    )


def _finalize(parts: np.ndarray):
    """parts: (ncores, 8) = [wv0, wv1, mlg0, mlg1, eall0, eall1, rep0, rep1]."""
    r = parts.astype(np.float64).sum(axis=0)
    disp = r[0] + r[1]
    ent = (r[2] + r[3]) / N
    rep = r[6] + r[7]
    inter = (r[4] + r[5] - F * K) / (2.0 * F)
    tot = disp + LAMBDA_ENTROPY * ent + LAMBDA_REPULSION * rep + LAMBDA_INTER * inter
    return tuple(np.float32(v) for v in (tot, disp, ent, rep, inter))


def _build_nc():
    import concourse.bacc as bacc
    import concourse.tile as tile
    from concourse import mybir

    f32 = mybir.dt.float32
    f16 = mybir.dt.float16
    f8 = mybir.dt.float8e4
    DR = mybir.MatmulPerfMode.DoubleRow
    AF = mybir.ActivationFunctionType
    OP = mybir.AluOpType

    nc = bacc.Bacc("TRN2", target_bir_lowering=False, debug=False,
                   enable_asserts=False, enable_partition_id=False)
    g_dram = nc.dram_tensor("g", (NGB * 128, GB * 2 * FK), f8, kind="ExternalInput").ap()
    y_dram = nc.dram_tensor("y", (128, NPAIR * 2 * W), f8, kind="ExternalInput").ap()
    out_dram = nc.dram_tensor("out", (1, 8), f32, kind="ExternalOutput").ap()

    with tile.TileContext(nc) as tc:
        with (
            tc.tile_pool(name="singles", bufs=1) as singles,
            tc.tile_pool(name="gpool", bufs=4) as gpool,
            tc.tile_pool(name="scr", bufs=2) as scr,
            tc.tile_pool(name="ph2", bufs=1) as ph2,
            tc.tile_pool(name="psacc", bufs=1, space="PSUM") as psacc,
            tc.tile_pool(name="pstmp", bufs=1, space="PSUM") as pstmp,
        ):
            yres = singles.tile([128, NPAIR * 2 * W], f8, name="yres")

            def emit_ychunk(ci):
                lo = ci * YCH * 2 * W
                hi = (ci + 1) * YCH * 2 * W
                nc.sync.dma_start(out=yres[:, lo:hi], in_=y_dram[:, lo:hi])

            # ---- constants ----
            ones128 = singles.tile([128, 1], f32)
            nc.gpsimd.memset(ones128, 1.0)
            eps128 = singles.tile([128, 1], f32)
            nc.gpsimd.memset(eps128, EPS)
            ones16c = singles.tile([128, 1], f16)
            nc.gpsimd.memset(ones16c, 1.0)
            id16 = singles.tile([128, 128], f16)
            nc.gpsimd.memset(id16, 0.0)
            nc.gpsimd.affine_select(
                out=id16, in_=id16,
                compare_op=OP.not_equal,
                fill=1.0, base=0, pattern=[[-1, 128]], channel_multiplier=1,
            )
            ones_row = singles.tile([1, FK], f16)
            nc.gpsimd.memset(ones_row, 1.0)
            qneg_sb = singles.tile([1, FK], f16)
            # repulsion masks (fp16): Mrep_h[p, j] = 1 iff j == 128h + p + 1
            # and (128h+p) % 32 != 31.  2D-pattern affine_select on the
            # j%32!=0 subview never touches feature-crossing columns.
            mrep = []
            for h in range(2):
                m = singles.tile([128, FK], f16, name=f"mrep{h}")
                nc.gpsimd.memset(m, 0.0)
                m3 = m.rearrange("p (blk c) -> p blk c", c=32)
                nc.gpsimd.affine_select(
                    out=m3[:, :, 1:32], in_=m3[:, :, 1:32],
                    compare_op=OP.not_equal,
                    fill=1.0, base=-128 * h, pattern=[[32, 8], [1, 31]],
                    channel_multiplier=-1,
                )
                mrep.append(m)

            # ---- preload Exp+Ln activation tables (2 slots); the tail uses
            # no other scalar-engine functions, so no reloads there.
            warm = ph2.tile([1, 2], f32)
            nc.scalar.activation(out=warm[0:1, 0:1], in_=ones128[0:1, 0:1], func=AF.Exp)
            nc.scalar.activation(out=warm[0:1, 1:2], in_=ones128[0:1, 0:1], func=AF.Ln)

            # ---- phase 1: G-stationary DoubleRow accumulation ----
            # single psum tile, two 132-col accumulation regions:
            # ps[:, h*W .. h*W+127]=SWY, +128=mass_raw, +129/130=A_hi/lo
            ps = psacc.tile([128, 2 * W], f32, name="acc")
            ps3 = ps.rearrange("p (h w) -> p h w", h=2)
            # sync-queue order == consumption order
            YS = {0: 0, 2: 1, 4: 2, 6: 3}
            for b in range(NGB):
                if b in YS:
                    emit_ychunk(YS[b])
                g = gpool.tile([128, GB * 2 * FK], f8)
                nc.sync.dma_start(out=g, in_=g_dram[b * 128:(b + 1) * 128, :])
                for ul in range(GB):
                    u = b * GB + ul
                    gv = g[:, ul * 2 * FK:(ul + 1) * 2 * FK].rearrange(
                        "p (i fk) -> p i fk", i=2)
                    yv = yres[:, u * 2 * W:(u + 1) * 2 * W].rearrange(
                        "p (i w) -> p i w", i=2)
                    for h in range(2):
                        nc.tensor.matmul(
                            ps[:, h * W:(h + 1) * W],
                            gv[:, :, h * 128:(h + 1) * 128], yv,
                            start=(u == 0), stop=(u == NPAIR - 1),
                            perf_mode=DR,
                        )

            # ---- tail ----
            # st cols: [wv0, wv1, mass*ln0, mass*ln1, eall0, eall1, rep0, rep1]
            st = ph2.tile([128, 8], f32)
            mass = ph2.tile([128, 2], f32)
            inv = ph2.tile([128, 2], f32)
            a_ = ph2.tile([128, 2], f32)
            csq = ph2.tile([128, 2], f32)
            t1 = ph2.tile([128, 2], f32)
            lg = ph2.tile([128, 2], f32)
            cent16 = ph2.tile([128, 2 * 128], f16)
            ccT = ph2.tile([128, FK], f16)
            nshift = ph2.tile([128, 1], f32)
            sqc = scr.tile([128, FK], f16, tag="sqc")
            sqf = scr.tile([128, FK], f16, tag="sqf")

            # vectorized per-bin stats ([128,2] = both halves at once)
            nc.vector.tensor_scalar_add(
                mass.rearrange("p (h one) -> p h one", one=1),
                in0=ps3[:, :, 128:129], scalar1=eps128)
            nc.vector.reduce_sum(a_, ps3[:, :, 129:131], axis=mybir.AxisListType.X)
            nc.vector.reciprocal(inv, mass)
            for h in range(2):
                with nc.allow_low_precision(reason="cent fp16 for exp terms"):
                    nc.vector.tensor_scalar_mul(
                        cent16[:, h * 128:(h + 1) * 128],
                        in0=ps[:, h * W:h * W + 128], scalar1=inv[:, h:h + 1],
                    )
            with nc.allow_low_precision(reason="csq via fp16 cent"):
                nc.vector.tensor_mul(sqc, cent16, cent16)
            nc.vector.reduce_sum(
                csq, sqc.rearrange("p (h c) -> p h c", c=128),
                axis=mybir.AxisListType.X)
            nc.vector.tensor_mul(t1, a_, inv)
            nc.vector.tensor_sub(st[:, 0:2], t1, csq)
            # entropy: st2/3 = mass * ln(mass/N + EPS); host divides by N
            nc.scalar.activation(out=lg, in_=mass, func=AF.Ln,
                                 bias=eps128, scale=1.0 / N)
            nc.vector.tensor_mul(st[:, 2:4], mass, lg)

            # transpose cent16 -> [c, fk]; center by bin 0 and scale x16
            psT = []
            for h in range(2):
                ps_t = pstmp.tile([128, 128], f32, tag=f"psT{h}", name=f"psT{h}")
                nc.tensor.matmul(ps_t, cent16[:, h * 128:(h + 1) * 128], id16,
                                 start=True, stop=True)
                psT.append(ps_t)
            nc.vector.tensor_scalar_mul(nshift, in0=psT[0][:, 0:1], scalar1=-1.0)
            for h in range(2):
                with nc.allow_low_precision(reason="cc fp16 for exp terms"):
                    nc.vector.tensor_scalar(
                        out=ccT[:, h * 128:(h + 1) * 128], in0=psT[h],
                        scalar1=nshift, scalar2=CSC, op0=OP.add, op1=OP.mult,
                    )
            # q_j = sum_c ccT^2 (column sums via fp16 ones-matmul)
            with nc.allow_low_precision(reason="scaled cc^2 fits fp16"):
                nc.vector.tensor_mul(sqf, ccT, ccT)
            ps_q = pstmp.tile([1, FK], f32, tag="psq")
            nc.tensor.matmul(ps_q, ones16c, sqf, start=True, stop=True)
            with nc.allow_low_precision(reason="q fp16 rank-1 operand"):
                nc.vector.tensor_scalar_mul(qneg_sb, in0=ps_q, scalar1=-0.5)

            # pairwise: psE = (dots - q_k/2 - q_j/2) * CSC^2 ; E = exp(2*psE/CSC^2)
            for h in range(2):
                ps_e = pstmp.tile([128, FK], f32, tag=f"psE{h}", name=f"psE{h}")
                nc.tensor.matmul(ps_e, ccT[:, h * 128:(h + 1) * 128], ccT,
                                 start=True, stop=False)
                nc.tensor.matmul(ps_e, ones_row[0:1, 0:128], qneg_sb,
                                 start=False, stop=False)
                nc.tensor.matmul(ps_e, qneg_sb[0:1, h * 128:(h + 1) * 128], ones_row,
                                 start=False, stop=True)
                e_full = scr.tile([128, FK], f16, tag="efull", name=f"ef{h}")
                with nc.allow_low_precision(reason="E<=1 fp16"):
                    nc.scalar.activation(out=e_full, in_=ps_e, func=AF.Exp,
                                         scale=2.0 / (CSC * CSC))
                # repulsion (gpsimd, parallel with the vector-engine reduces)
                edump = scr.tile([128, FK], f16, tag="edump", name=f"ed{h}")
                with nc.allow_low_precision(reason="masked E fp16"):
                    nc.gpsimd.tensor_mul(edump, e_full, mrep[h])
                nc.vector.reduce_sum(st[:, 6 + h:7 + h], edump,
                                     axis=mybir.AxisListType.X)
                # inter: same-feature 32x32 block row-sums (vector)
                for fl in range(4):
                    fg = h * 4 + fl
                    nc.vector.reduce_sum(
                        st[32 * fl:32 * fl + 32, 4 + h:5 + h],
                        e_full[32 * fl:32 * fl + 32, fg * 32:fg * 32 + 32],
                        axis=mybir.AxisListType.X,
                    )

            ps_res = pstmp.tile([1, 8], f32, tag="psres")
            nc.tensor.matmul(ps_res, ones128, st, start=True, stop=True)
            res = ph2.tile([1, 8], f32)
            nc.vector.tensor_copy(res, ps_res)
            nc.sync.dma_start(out=out_dram, in_=res)

    nc.compile()
    return nc


def get_nc():
    if "v3" not in _NC_CACHE:
        _NC_CACHE["v3"] = _build_nc()
    return _NC_CACHE["v3"]


def kernel(membership: np.ndarray, teacher_preds: np.ndarray, _trace: bool = False):
    from concourse.bass_utils import run_bass_kernel_spmd

    f8 = _f8dtype()
    m = np.asarray(membership, dtype=np.float32).reshape(N, F * K)
    y32 = np.asarray(teacher_preds, dtype=np.float32)
    ysq = np.einsum("nc,nc->n", y32, y32, dtype=np.float64).astype(np.float32)
    ysq_h = ysq.astype(f8)
    ysq_l = (ysq - ysq_h.astype(np.float32)).astype(f8)
    yslab = np.zeros((N, W), dtype=f8)
    yslab[:, 0:C] = y32.astype(f8)
    yslab[:, C] = np.float32(1.0)
    yslab[:, C + 1] = ysq_h
    yslab[:, C + 2] = ysq_l
    ypacked = _pack_y(yslab)

    m8 = m.astype(f8)
    nc = get_nc()
    in_maps = []
    for i in range(NCORES):
        in_maps.append({
            "g": _pack_g(m8[:, i * FK:(i + 1) * FK]),
            "y": ypacked,
        })
    res = run_bass_kernel_spmd(
        nc, in_maps, core_ids=list(range(NCORES)), trace=_trace,
    )
    parts = np.stack(
        [np.asarray(res.results[i]["out"][0], dtype=np.float64) for i in range(NCORES)]
    )
    out = _finalize(parts)
    if _trace:
        return out, res
    return out


if __name__ == "__main__":
    rng = np.random.default_rng(0)
    mem = rng.random((N, F, K), dtype=np.float32)
    tp = rng.random((N, C), dtype=np.float32)
    print(kernel(mem, tp))


# revision 18
# speedup vs baseline: 1.7968x; 1.0377x over previous
"""DispersionLoss kernel for Trainium2 (8 NeuronCores, Bass/Tile).

Reference computation (N=16384, F=64, K=32, C=128):
    bin_mass[f,k]  = sum_n m[n,f,k] + EPS
    SWY[f,k,c]     = sum_n m[n,f,k] * y[n,c]
    cent[f,k,c]    = SWY / bin_mass
    loss_dispersion= sum_fk (A/bin_mass - c_sq)   [EPS*c_sq/bin_mass ~1e-11, dropped]
        where A[f,k] = sum_n m[n,f,k]*|y_n|^2
    loss_entropy   = sum_fk p*log(p+EPS), p = bin_mass/N
    loss_repulsion = sum_f sum_k exp(-|cent[f,k]-cent[f,k+1]|^2)
    loss_inter     = sum_f (sum_{kj} exp(-pairwise) - K) / 2 / F

Sharding: over F (8 features per core) -> every loss term decomposes per-f,
no cross-core collectives; host sums 8 partial vectors.

Design:
  - inputs quantized to fp8 e4m3 on host; ysq precomputed on host (f32) and
    shipped as an fp8 hi+lo pair -> device does zero prep work.  ~6.1 MiB/core.
  - all input DMA on the sync queue in consumption order (y chunks
    interleaved between g blocks); gpsimd's software-DGE path is avoided.
  - G-stationary DoubleRow fp8 matmuls: for each 256-row pair u and bin-half
    h, psum[:, h*W:(h+1)*W] += g[u,:,h-half].T @ [Y | 1 | ysq_h | ysq_l].
    G enters the PE once; mass/A ride along as extra moving columns; output
    is bin-major so no transpose stage is needed for the per-bin stats.
  - tail: only Ln/Exp touch the scalar engine (tables preloaded at start; no
    ACT_TABLE_LOAD stalls); everything else on vector/gpsimd/PE.  Centroids
    are centered and scaled x16 so all fp16 pairwise math is in normal range.
"""

import numpy as np

N = 16384
F = 64
K = 32
C = 128
NCORES = 8
F_PER_CORE = F // NCORES          # 8
FK = F_PER_CORE * K               # 256 bins per core
NPAIR = N // 256                  # 64 subtile pairs (DoubleRow: 256 rows/mm)
W = 132                           # moving cols: [y(128) | 1 | ysq_h | ysq_l | pad]
GB = 8                            # pairs per g DMA block
NGB = NPAIR // GB                 # 8 g blocks
YCH = 16                          # pairs per y DMA chunk
NYCH = NPAIR // YCH               # 4 y chunks
CSC = 16.0                        # centered-centroid scale (keeps fp16 normal)

LAMBDA_ENTROPY = 0.1
LAMBDA_REPULSION = 0.5
LAMBDA_INTER = 0.3
EPS = 1e-8

_NC_CACHE = {}


def _f8dtype():
    import ml_dtypes
    return ml_dtypes.float8_e4m3


def _pack_g(gc: np.ndarray) -> np.ndarray:
    """(N, FK) fp8 -> (NGB*128, GB*2*FK): block b row p holds, for the 8
    pairs u of the block, [i=0 | i=1] x FK cols where the n-row is
    256*u + 128*i + p."""
    x = gc.reshape(NPAIR, 2, 128, FK).transpose(2, 0, 1, 3)   # p, u, i, fk
    x = x.reshape(128, NPAIR * 2 * FK).reshape(128, NGB, GB * 2 * FK)
    return np.ascontiguousarray(x.transpose(1, 0, 2).reshape(NGB * 128, GB * 2 * FK))


def _pack_y(yslab: np.ndarray) -> np.ndarray:
    """(N, W) fp8 -> (128, NPAIR*2*W): partition p holds pair-major slabs."""
    return np.ascontiguousarray(
        yslab.reshape(NPAIR, 2, 128, W).transpose(2, 0, 1, 3).reshape(128, NPAIR * 2 * W)
    )


def _finalize(parts: np.ndarray):
    """parts: (ncores, 8) = [wv0, wv1, mlg0, mlg1, eall0, eall1, rep0, rep1]."""
    r = parts.astype(np.float64).sum(axis=0)
    disp = r[0] + r[1]
    ent = (r[2] + r[3]) / N
    rep = r[6] + r[7]
    inter = (r[4] + r[5] - F * K) / (2.0 * F)
    tot = disp + LAMBDA_ENTROPY * ent + LAMBDA_REPULSION * rep + LAMBDA_INTER * inter
    return tuple(np.float32(v) for v in (tot, disp, ent, rep, inter))


def _build_nc():
    import concourse.bacc as bacc
    import concourse.tile as tile
    from concourse import mybir

    f32 = mybir.dt.float32
    f16 = mybir.dt.float16
    f8 = mybir.dt.float8e4
    DR = mybir.MatmulPerfMode.DoubleRow
    AF = mybir.ActivationFunctionType
    OP = mybir.AluOpType

    nc = bacc.Bacc("TRN2", target_bir_lowering=False, debug=False,
                   enable_asserts=False, enable_partition_id=False)
    g_dram = nc.dram_tensor("g", (NGB * 128, GB * 2 * FK), f8, kind="ExternalInput").ap()
    y_dram = nc.dram_tensor("y", (128, NPAIR * 2 * W), f8, kind="ExternalInput").ap()
    out_dram = nc.dram_tensor("out", (1, 8), f32, kind="ExternalOutput").ap()

    with tile.TileContext(nc) as tc:
        with (
            tc.tile_pool(name="singles", bufs=1) as singles,
            tc.tile_pool(name="gpool", bufs=4) as gpool,
            tc.tile_pool(name="scr", bufs=2) as scr,
            tc.tile_pool(name="ph2", bufs=1) as ph2,
            tc.tile_pool(name="psacc", bufs=1, space="PSUM") as psacc,
            tc.tile_pool(name="pstmp", bufs=1, space="PSUM") as pstmp,
        ):
            yres = singles.tile([128, NPAIR * 2 * W], f8, name="yres")

            def emit_ychunk(ci):
                lo = ci * YCH * 2 * W
                hi = (ci + 1) * YCH * 2 * W
                nc.sync.dma_start(out=yres[:, lo:hi], in_=y_dram[:, lo:hi])

            # ---- constants ----
            ones128 = singles.tile([128, 1], f32)
            nc.gpsimd.memset(ones128, 1.0)
            eps128 = singles.tile([128, 1], f32)
            nc.gpsimd.memset(eps128, EPS)
            ones16c = singles.tile([128, 1], f16)
            nc.gpsimd.memset(ones16c, 1.0)
            id16 = singles.tile([128, 128], f16)
            nc.gpsimd.memset(id16, 0.0)
            nc.gpsimd.affine_select(
                out=id16, in_=id16,
                compare_op=OP.not_equal,
                fill=1.0, base=0, pattern=[[-1, 128]], channel_multiplier=1,
            )
            ones_row = singles.tile([1, FK], f16)
            nc.gpsimd.memset(ones_row, 1.0)
            qneg_sb = singles.tile([1, FK], f16)
            # repulsion masks (fp16): Mrep_h[p, j] = 1 iff j == 128h + p + 1
            # and (128h+p) % 32 != 31.  2D-pattern affine_select on the
            # j%32!=0 subview never touches feature-crossing columns.
            mrep = []
            for h in range(2):
                m = singles.tile([128, FK], f16, name=f"mrep{h}")
                nc.gpsimd.memset(m, 0.0)
                m3 = m.rearrange("p (blk c) -> p blk c", c=32)
                nc.gpsimd.affine_select(
                    out=m3[:, :, 1:32], in_=m3[:, :, 1:32],
                    compare_op=OP.not_equal,
                    fill=1.0, base=-128 * h, pattern=[[32, 8], [1, 31]],
                    channel_multiplier=-1,
                )
                mrep.append(m)

            # ---- preload Exp+Ln activation tables (2 slots); the tail uses
            # no other scalar-engine functions, so no reloads there.
            warm = ph2.tile([1, 2], f32)
            nc.scalar.activation(out=warm[0:1, 0:1], in_=ones128[0:1, 0:1], func=AF.Exp)
            nc.scalar.activation(out=warm[0:1, 1:2], in_=ones128[0:1, 0:1], func=AF.Ln)

            # ---- phase 1: G-stationary DoubleRow accumulation ----
            # single psum tile, two 132-col accumulation regions:
            # ps[:, h*W .. h*W+127]=SWY, +128=mass_raw, +129/130=A_hi/lo
            ps = psacc.tile([128, 2 * W], f32, name="acc")
            ps3 = ps.rearrange("p (h w) -> p h w", h=2)
            # sync-queue order == consumption order
            YS = {0: 0, 2: 1, 4: 2, 6: 3}
            for b in range(NGB):
                if b in YS:
                    emit_ychunk(YS[b])
                g = gpool.tile([128, GB * 2 * FK], f8)
                nc.sync.dma_start(out=g, in_=g_dram[b * 128:(b + 1) * 128, :])
                for ul in range(GB):
                    u = b * GB + ul
                    gv = g[:, ul * 2 * FK:(ul + 1) * 2 * FK].rearrange(
                        "p (i fk) -> p i fk", i=2)
                    yv = yres[:, u * 2 * W:(u + 1) * 2 * W].rearrange(
                        "p (i w) -> p i w", i=2)
                    for h in range(2):
                        nc.tensor.matmul(
                            ps[:, h * W:(h + 1) * W],
                            gv[:, :, h * 128:(h + 1) * 128], yv,
                            start=(u == 0), stop=(u == NPAIR - 1),
                            perf_mode=DR,
                        )

            # ---- tail ----
            # st cols: [wv0, wv1, mass*ln0, mass*ln1, eall0, eall1, rep0, rep1]
            st = ph2.tile([128, 8], f32)
            mass = ph2.tile([128, 2], f32)
            inv = ph2.tile([128, 2], f32)
            a_ = ph2.tile([128, 2], f32)
            csq = ph2.tile([128, 2], f32)
            t1 = ph2.tile([128, 2], f32)
            lg = ph2.tile([128, 2], f32)
            cent16 = ph2.tile([128, 2 * 128], f16)
            ccT = ph2.tile([128, FK], f16)
            nshift = ph2.tile([128, 1], f32)
            sqc = scr.tile([128, FK], f16, tag="sqc")
            sqf = scr.tile([128, FK], f16, tag="sqf")

            # vectorized per-bin stats ([128,2] = both halves at once)
            nc.vector.tensor_scalar_add(
                mass.rearrange("p (h one) -> p h one", one=1),
                in0=ps3[:, :, 128:129], scalar1=eps128)
            nc.vector.reduce_sum(a_, ps3[:, :, 129:131], axis=mybir.AxisListType.X)
            nc.vector.reciprocal(inv, mass)
            for h in range(2):
                with nc.allow_low_precision(reason="cent fp16 for exp terms"):
                    nc.vector.tensor_scalar_mul(
                        cent16[:, h * 128:(h + 1) * 128],
                        in0=ps[:, h * W:h * W + 128], scalar1=inv[:, h:h + 1],
                    )
            with nc.allow_low_precision(reason="csq via fp16 cent"):
                nc.vector.tensor_mul(sqc, cent16, cent16)
            nc.vector.reduce_sum(
                csq, sqc.rearrange("p (h c) -> p h c", c=128),
                axis=mybir.AxisListType.X)
            nc.vector.tensor_mul(t1, a_, inv)
            nc.vector.tensor_sub(st[:, 0:2], t1, csq)
            # entropy: st2/3 = mass * ln(mass/N + EPS); host divides by N
            for h in range(2):
                nc.scalar.activation(out=lg[:, h:h + 1], in_=mass[:, h:h + 1],
                                     func=AF.Ln, bias=eps128, scale=1.0 / N)
            nc.vector.tensor_mul(st[:, 2:4], mass, lg)

            # transpose cent16 -> [c, fk]; center by bin 0 and scale x16
            psT = []
            for h in range(2):
                ps_t = pstmp.tile([128, 128], f32, tag=f"psT{h}", name=f"psT{h}")
                nc.tensor.matmul(ps_t, cent16[:, h * 128:(h + 1) * 128], id16,
                                 start=True, stop=True)
                psT.append(ps_t)
            nc.vector.tensor_scalar_mul(nshift, in0=psT[0][:, 0:1], scalar1=-1.0)
            for h in range(2):
                with nc.allow_low_precision(reason="cc fp16 for exp terms"):
                    nc.vector.tensor_scalar(
                        out=ccT[:, h * 128:(h + 1) * 128], in0=psT[h],
                        scalar1=nshift, scalar2=CSC, op0=OP.add, op1=OP.mult,
                    )
            # q_j = sum_c ccT^2 (column sums via fp16 ones-matmul)
            with nc.allow_low_precision(reason="scaled cc^2 fits fp16"):
                nc.vector.tensor_mul(sqf, ccT, ccT)
            ps_q = pstmp.tile([1, FK], f32, tag="psq")
            nc.tensor.matmul(ps_q, ones16c, sqf, start=True, stop=True)
            with nc.allow_low_precision(reason="q fp16 rank-1 operand"):
                nc.vector.tensor_scalar_mul(qneg_sb, in0=ps_q, scalar1=-0.5)

            # pairwise: psE = (dots - q_k/2 - q_j/2) * CSC^2 ; E = exp(2*psE/CSC^2)
            for h in range(2):
                ps_e = pstmp.tile([128, FK], f32, tag=f"psE{h}", name=f"psE{h}")
                nc.tensor.matmul(ps_e, ccT[:, h * 128:(h + 1) * 128], ccT,
                                 start=True, stop=False)
                nc.tensor.matmul(ps_e, ones_row[0:1, 0:128], qneg_sb,
                                 start=False, stop=False)
                nc.tensor.matmul(ps_e, qneg_sb[0:1, h * 128:(h + 1) * 128], ones_row,
                                 start=False, stop=True)
                e_full = scr.tile([128, FK], f16, tag="efull", name=f"ef{h}")
                with nc.allow_low_precision(reason="E<=1 fp16"):
                    nc.scalar.activation(out=e_full, in_=ps_e, func=AF.Exp,
                                         scale=2.0 / (CSC * CSC))
                # repulsion (gpsimd, parallel with the vector-engine reduces)
                edump = scr.tile([128, FK], f16, tag="edump", name=f"ed{h}")
                with nc.allow_low_precision(reason="masked E fp16"):
                    nc.gpsimd.tensor_mul(edump, e_full, mrep[h])
                nc.vector.reduce_sum(st[:, 6 + h:7 + h], edump,
                                     axis=mybir.AxisListType.X)
                # inter: same-feature 32x32 block row-sums (vector)
                for fl in range(4):
                    fg = h * 4 + fl
                    nc.vector.reduce_sum(
                        st[32 * fl:32 * fl + 32, 4 + h:5 + h],
                        e_full[32 * fl:32 * fl + 32, fg * 32:fg * 32 + 32],
                        axis=mybir.AxisListType.X,
                    )

            ps_res = pstmp.tile([1, 8], f32, tag="psres")
            nc.tensor.matmul(ps_res, ones128, st, start=True, stop=True)
            res = ph2.tile([1, 8], f32)
            nc.vector.tensor_copy(res, ps_res)
            nc.sync.dma_start(out=out_dram, in_=res)

    nc.compile()
    return nc


def get_nc():
    if "v3" not in _NC_CACHE:
        _NC_CACHE["v3"] = _build_nc()
    return _NC_CACHE["v3"]


def kernel(membership: np.ndarray, teacher_preds: np.ndarray, _trace: bool = False):
    from concourse.bass_utils import run_bass_kernel_spmd

    f8 = _f8dtype()
    m = np.asarray(membership, dtype=np.float32).reshape(N, F * K)
    y32 = np.asarray(teacher_preds, dtype=np.float32)
    ysq = np.einsum("nc,nc->n", y32, y32, dtype=np.float64).astype(np.float32)
    ysq_h = ysq.astype(f8)
    ysq_l = (ysq - ysq_h.astype(np.float32)).astype(f8)
    yslab = np.zeros((N, W), dtype=f8)
    yslab[:, 0:C] = y32.astype(f8)
    yslab[:, C] = np.float32(1.0)
    yslab[:, C + 1] = ysq_h
    yslab[:, C + 2] = ysq_l
    ypacked = _pack_y(yslab)

    m8 = m.astype(f8)
    nc = get_nc()
    in_maps = []
    for i in range(NCORES):
        in_maps.append({
            "g": _pack_g(m8[:, i * FK:(i + 1) * FK]),
            "y": ypacked,
        })
    res = run_bass_kernel_spmd(
        nc, in_maps, core_ids=list(range(NCORES)), trace=_trace,
    )
    parts = np.stack(
        [np.asarray(res.results[i]["out"][0], dtype=np.float64) for i in range(NCORES)]
    )
    out = _finalize(parts)
    if _trace:
        return out, res
    return out


if __name__ == "__main__":
    rng = np.random.default_rng(0)
    mem = rng.random((N, F, K), dtype=np.float32)
    tp = rng.random((N, C), dtype=np.float32)
    print(kernel(mem, tp))


# revision 20
# speedup vs baseline: 1.9759x; 1.0997x over previous
"""DispersionLoss kernel for Trainium2 (8 NeuronCores, Bass/Tile).

Reference computation (N=16384, F=64, K=32, C=128):
    bin_mass[f,k]  = sum_n m[n,f,k] + EPS
    SWY[f,k,c]     = sum_n m[n,f,k] * y[n,c]
    cent[f,k,c]    = SWY / bin_mass
    loss_dispersion= sum_fk (A/bin_mass - c_sq)   [EPS*c_sq/bin_mass ~1e-11, dropped]
        where A[f,k] = sum_n m[n,f,k]*|y_n|^2
    loss_entropy   = sum_fk p*log(p+EPS), p = bin_mass/N
    loss_repulsion = sum_f sum_k exp(-|cent[f,k]-cent[f,k+1]|^2)
    loss_inter     = sum_f (sum_{kj} exp(-pairwise) - K) / 2 / F

Sharding: over F (8 features per core) -> every loss term decomposes per-f,
no cross-core collectives; host sums 8 partial vectors.

Design:
  - inputs quantized to fp8 e4m3 on host; ysq precomputed on host (f32) and
    shipped as an fp8 hi+lo pair -> device does zero prep work.  ~6.1 MiB/core.
  - all input DMA on the sync queue in consumption order (y chunks
    interleaved between g blocks); gpsimd's software-DGE path is avoided.
  - G-stationary DoubleRow fp8 matmuls: for each 256-row pair u and bin-half
    h, psum[:, h*W:(h+1)*W] += g[u,:,h-half].T @ [Y | 1 | ysq_h | ysq_l].
    G enters the PE once; mass/A ride along as extra moving columns; output
    is bin-major so no transpose stage is needed for the per-bin stats.
  - tail: only Ln/Exp touch the scalar engine (tables preloaded at start; no
    ACT_TABLE_LOAD stalls); everything else on vector/gpsimd/PE.  Centroids
    are centered and scaled x16 so all fp16 pairwise math is in normal range.
"""

import numpy as np

N = 16384
F = 64
K = 32
C = 128
NCORES = 8
F_PER_CORE = F // NCORES          # 8
FK = F_PER_CORE * K               # 256 bins per core
NPAIR = N // 256                  # 64 subtile pairs (DoubleRow: 256 rows/mm)
W = 132                           # moving cols: [y(128) | 1 | ysq_h | ysq_l | pad]
GB = 8                            # pairs per g DMA block
NGB = NPAIR // GB                 # 8 g blocks
YCH = 16                          # pairs per y DMA chunk
NYCH = NPAIR // YCH               # 4 y chunks
CSC = 16.0                        # centered-centroid scale (keeps fp16 normal)

LAMBDA_ENTROPY = 0.1
LAMBDA_REPULSION = 0.5
LAMBDA_INTER = 0.3
EPS = 1e-8

_NC_CACHE = {}


def _f8dtype():
    import ml_dtypes
    return ml_dtypes.float8_e4m3


def _pack_g(gc: np.ndarray) -> np.ndarray:
    """(N, FK) fp8 -> (NGB*128, GB*2*FK): block b row p holds, for the 8
    pairs u of the block, [i=0 | i=1] x FK cols where the n-row is
    256*u + 128*i + p."""
    x = gc.reshape(NPAIR, 2, 128, FK).transpose(2, 0, 1, 3)   # p, u, i, fk
    x = x.reshape(128, NPAIR * 2 * FK).reshape(128, NGB, GB * 2 * FK)
    return np.ascontiguousarray(x.transpose(1, 0, 2).reshape(NGB * 128, GB * 2 * FK))


def _pack_y(yslab: np.ndarray) -> np.ndarray:
    """(N, W) fp8 -> (128, NPAIR*2*W): partition p holds pair-major slabs."""
    return np.ascontiguousarray(
        yslab.reshape(NPAIR, 2, 128, W).transpose(2, 0, 1, 3).reshape(128, NPAIR * 2 * W)
    )


def _finalize(parts: np.ndarray):
    """parts: (ncores, 8) = [wv0, wv1, mlg0, mlg1, eall0, eall1, rep0, rep1]."""
    r = parts.astype(np.float64).sum(axis=0)
    disp = r[0] + r[1]
    ent = (r[2] + r[3]) / N
    rep = r[6] + r[7]
    inter = (r[4] + r[5] - F * K) / (2.0 * F)
    tot = disp + LAMBDA_ENTROPY * ent + LAMBDA_REPULSION * rep + LAMBDA_INTER * inter
    return tuple(np.float32(v) for v in (tot, disp, ent, rep, inter))


def _build_nc():
    import concourse.bacc as bacc
    import concourse.tile as tile
    from concourse import mybir

    f32 = mybir.dt.float32
    f16 = mybir.dt.float16
    f8 = mybir.dt.float8e4
    DR = mybir.MatmulPerfMode.DoubleRow
    AF = mybir.ActivationFunctionType
    OP = mybir.AluOpType

    nc = bacc.Bacc("TRN2", target_bir_lowering=False, debug=False,
                   enable_asserts=False, enable_partition_id=False)
    g_dram = nc.dram_tensor("g", (NGB * 128, GB * 2 * FK), f8, kind="ExternalInput").ap()
    y_dram = nc.dram_tensor("y", (128, NPAIR * 2 * W), f8, kind="ExternalInput").ap()
    out_dram = nc.dram_tensor("out", (1, 8), f32, kind="ExternalOutput").ap()

    with tile.TileContext(nc) as tc:
        with (
            tc.tile_pool(name="singles", bufs=1) as singles,
            tc.tile_pool(name="gpool", bufs=4) as gpool,
            tc.tile_pool(name="scr", bufs=2) as scr,
            tc.tile_pool(name="ph2", bufs=1) as ph2,
            tc.tile_pool(name="psacc", bufs=1, space="PSUM") as psacc,
            tc.tile_pool(name="pstmp", bufs=1, space="PSUM") as pstmp,
        ):
            yres = singles.tile([128, NPAIR * 2 * W], f8, name="yres")

            def emit_ychunk(ci):
                lo = ci * YCH * 2 * W
                hi = (ci + 1) * YCH * 2 * W
                nc.sync.dma_start(out=yres[:, lo:hi], in_=y_dram[:, lo:hi])

            # ---- constants ----
            ones128 = singles.tile([128, 1], f32)
            nc.gpsimd.memset(ones128, 1.0)
            eps128 = singles.tile([128, 1], f32)
            nc.gpsimd.memset(eps128, EPS)
            ones16c = singles.tile([128, 1], f16)
            nc.gpsimd.memset(ones16c, 1.0)
            id16 = singles.tile([128, 128], f16)
            nc.gpsimd.memset(id16, 0.0)
            nc.gpsimd.affine_select(
                out=id16, in_=id16,
                compare_op=OP.not_equal,
                fill=1.0, base=0, pattern=[[-1, 128]], channel_multiplier=1,
            )
            ones_row = singles.tile([1, FK], f16)
            nc.gpsimd.memset(ones_row, 1.0)
            qneg_sb = singles.tile([1, FK], f16)
            # repulsion masks (fp16): Mrep_h[p, j] = 1 iff j == 128h + p + 1
            # and (128h+p) % 32 != 31.  2D-pattern affine_select on the
            # j%32!=0 subview never touches feature-crossing columns.
            mrep = []
            for h in range(2):
                m = singles.tile([128, FK], f16, name=f"mrep{h}")
                nc.gpsimd.memset(m, 0.0)
                m3 = m.rearrange("p (blk c) -> p blk c", c=32)
                nc.gpsimd.affine_select(
                    out=m3[:, :, 1:32], in_=m3[:, :, 1:32],
                    compare_op=OP.not_equal,
                    fill=1.0, base=-128 * h, pattern=[[32, 8], [1, 31]],
                    channel_multiplier=-1,
                )
                mrep.append(m)

            # ---- preload Exp+Ln activation tables (2 slots); the tail uses
            # no other scalar-engine functions, so no reloads there.
            warm = ph2.tile([1, 2], f32)
            nc.scalar.activation(out=warm[0:1, 0:1], in_=ones128[0:1, 0:1], func=AF.Exp)
            nc.scalar.activation(out=warm[0:1, 1:2], in_=ones128[0:1, 0:1], func=AF.Ln)

            # ---- phase 1: G-stationary DoubleRow accumulation ----
            # two psum tiles (interleaved accumulation groups sharing one
            # tile silently drop partial sums):
            # ps[h][:, 0:128]=SWY, [:,128]=mass_raw, [:,129:131]=A_hi/lo
            ps = [psacc.tile([128, W], f32, name=f"acc{h}") for h in range(2)]
            # sync-queue order == consumption order
            YS = {0: 0, 2: 1, 4: 2, 6: 3}
            for b in range(NGB):
                if b in YS:
                    emit_ychunk(YS[b])
                g = gpool.tile([128, GB * 2 * FK], f8)
                nc.sync.dma_start(out=g, in_=g_dram[b * 128:(b + 1) * 128, :])
                for ul in range(GB):
                    u = b * GB + ul
                    gv = g[:, ul * 2 * FK:(ul + 1) * 2 * FK].rearrange(
                        "p (i fk) -> p i fk", i=2)
                    yv = yres[:, u * 2 * W:(u + 1) * 2 * W].rearrange(
                        "p (i w) -> p i w", i=2)
                    for h in range(2):
                        nc.tensor.matmul(
                            ps[h], gv[:, :, h * 128:(h + 1) * 128], yv,
                            start=(u == 0), stop=(u == NPAIR - 1),
                            perf_mode=DR,
                        )

            # ---- tail ----
            # st cols: [wv0, wv1, mass*ln0, mass*ln1, eall0, eall1, rep0, rep1]
            st = ph2.tile([128, 8], f32)
            mass = ph2.tile([128, 2], f32)
            inv = ph2.tile([128, 2], f32)
            a_ = ph2.tile([128, 2], f32)
            csq = ph2.tile([128, 2], f32)
            t1 = ph2.tile([128, 2], f32)
            lg = ph2.tile([128, 2], f32)
            cent16 = ph2.tile([128, 2 * 128], f16)
            ccT = ph2.tile([128, FK], f16)
            nshift = ph2.tile([128, 1], f32)
            sqc = scr.tile([128, FK], f16, tag="sqc")
            sqf = scr.tile([128, FK], f16, tag="sqf")

            # per-bin stats (both halves packed as [128,2] columns)
            for h in range(2):
                nc.vector.tensor_scalar_add(
                    mass[:, h:h + 1], in0=ps[h][:, 128:129], scalar1=eps128)
                nc.vector.reduce_sum(
                    a_[:, h:h + 1],
                    ps[h][:, 129:131].rearrange("p (one c) -> p one c", one=1),
                    axis=mybir.AxisListType.X)
            nc.vector.reciprocal(inv, mass)
            for h in range(2):
                with nc.allow_low_precision(reason="cent fp16 for exp terms"):
                    nc.vector.tensor_scalar_mul(
                        cent16[:, h * 128:(h + 1) * 128],
                        in0=ps[h][:, 0:128], scalar1=inv[:, h:h + 1],
                    )
            with nc.allow_low_precision(reason="csq via fp16 cent"):
                nc.vector.tensor_mul(sqc, cent16, cent16)
            nc.vector.reduce_sum(
                csq, sqc.rearrange("p (h c) -> p h c", c=128),
                axis=mybir.AxisListType.X)
            nc.vector.tensor_mul(t1, a_, inv)
            nc.vector.tensor_sub(st[:, 0:2], t1, csq)
            # entropy: st2/3 = mass * ln(mass/N + EPS); host divides by N
            for h in range(2):
                nc.scalar.activation(out=lg[:, h:h + 1], in_=mass[:, h:h + 1],
                                     func=AF.Ln, bias=eps128, scale=1.0 / N)
            nc.vector.tensor_mul(st[:, 2:4], mass, lg)

            # transpose cent16 -> [c, fk]; center by bin 0 and scale x16
            psT = []
            for h in range(2):
                ps_t = pstmp.tile([128, 128], f32, tag=f"psT{h}", name=f"psT{h}")
                nc.tensor.matmul(ps_t, cent16[:, h * 128:(h + 1) * 128], id16,
                                 start=True, stop=True)
                psT.append(ps_t)
            nc.vector.tensor_scalar_mul(nshift, in0=psT[0][:, 0:1], scalar1=-1.0)
            for h in range(2):
                with nc.allow_low_precision(reason="cc fp16 for exp terms"):
                    nc.vector.tensor_scalar(
                        out=ccT[:, h * 128:(h + 1) * 128], in0=psT[h],
                        scalar1=nshift, scalar2=CSC, op0=OP.add, op1=OP.mult,
                    )
            # q_j = sum_c ccT^2 (column sums via fp16 ones-matmul)
            with nc.allow_low_precision(reason="scaled cc^2 fits fp16"):
                nc.vector.tensor_mul(sqf, ccT, ccT)
            ps_q = pstmp.tile([1, FK], f32, tag="psq")
            nc.tensor.matmul(ps_q, ones16c, sqf, start=True, stop=True)
            with nc.allow_low_precision(reason="q fp16 rank-1 operand"):
                nc.vector.tensor_scalar_mul(qneg_sb, in0=ps_q, scalar1=-0.5)

            # pairwise: psE = (dots - q_k/2 - q_j/2) * CSC^2 ; E = exp(2*psE/CSC^2)
            for h in range(2):
                ps_e = pstmp.tile([128, FK], f32, tag=f"psE{h}", name=f"psE{h}")
                nc.tensor.matmul(ps_e, ccT[:, h * 128:(h + 1) * 128], ccT,
                                 start=True, stop=False)
                nc.tensor.matmul(ps_e, ones_row[0:1, 0:128], qneg_sb,
                                 start=False, stop=False)
                nc.tensor.matmul(ps_e, qneg_sb[0:1, h * 128:(h + 1) * 128], ones_row,
                                 start=False, stop=True)
                e_full = scr.tile([128, FK], f16, tag="efull", name=f"ef{h}")
                with nc.allow_low_precision(reason="E<=1 fp16"):
                    nc.scalar.activation(out=e_full, in_=ps_e, func=AF.Exp,
                                         scale=2.0 / (CSC * CSC))
                # repulsion (gpsimd, parallel with the vector-engine reduces)
                edump = scr.tile([128, FK], f16, tag="edump", name=f"ed{h}")
                with nc.allow_low_precision(reason="masked E fp16"):
                    nc.gpsimd.tensor_mul(edump, e_full, mrep[h])
                nc.vector.reduce_sum(st[:, 6 + h:7 + h], edump,
                                     axis=mybir.AxisListType.X)
                # inter: same-feature 32x32 block row-sums (vector)
                for fl in range(4):
                    fg = h * 4 + fl
                    nc.vector.reduce_sum(
                        st[32 * fl:32 * fl + 32, 4 + h:5 + h],
                        e_full[32 * fl:32 * fl + 32, fg * 32:fg * 32 + 32],
                        axis=mybir.AxisListType.X,
                    )

            ps_res = pstmp.tile([1, 8], f32, tag="psres")
            nc.tensor.matmul(ps_res, ones128, st, start=True, stop=True)
            res = ph2.tile([1, 8], f32)
            nc.vector.tensor_copy(res, ps_res)
            nc.sync.dma_start(out=out_dram, in_=res)

    nc.compile()
    return nc


def get_nc():
    if "v3" not in _NC_CACHE:
        _NC_CACHE["v3"] = _build_nc()
    return _NC_CACHE["v3"]


def kernel(membership: np.ndarray, teacher_preds: np.ndarray, _trace: bool = False):
    from concourse.bass_utils import run_bass_kernel_spmd

    f8 = _f8dtype()
    m = np.asarray(membership, dtype=np.float32).reshape(N, F * K)
    y32 = np.asarray(teacher_preds, dtype=np.float32)
    ysq = np.einsum("nc,nc->n", y32, y32, dtype=np.float64).astype(np.float32)
    ysq_h = ysq.astype(f8)
    ysq_l = (ysq - ysq_h.astype(np.float32)).astype(f8)
    yslab = np.zeros((N, W), dtype=f8)
    yslab[:, 0:C] = y32.astype(f8)
    yslab[:, C] = np.float32(1.0)
    yslab[:, C + 1] = ysq_h
    yslab[:, C + 2] = ysq_l
    ypacked = _pack_y(yslab)

    m8 = m.astype(f8)
    nc = get_nc()
    in_maps = []
    for i in range(NCORES):
        in_maps.append({
            "g": _pack_g(m8[:, i * FK:(i + 1) * FK]),
            "y": ypacked,
        })
    res = run_bass_kernel_spmd(
        nc, in_maps, core_ids=list(range(NCORES)), trace=_trace,
    )
    parts = np.stack(
        [np.asarray(res.results[i]["out"][0], dtype=np.float64) for i in range(NCORES)]
    )
    out = _finalize(parts)
    if _trace:
        return out, res
    return out


if __name__ == "__main__":
    rng = np.random.default_rng(0)
    mem = rng.random((N, F, K), dtype=np.float32)
    tp = rng.random((N, C), dtype=np.float32)
    print(kernel(mem, tp))
